# revision 18
# baseline (speedup 1.0000x reference)
"""Kimi-K2.5 tensorized MoE kernel for 8 TRN2 NeuronCores.

Sharding: expert-parallel. Core c owns routed experts [4c, 4c+4) and rows
[128c, 128(c+1)) of the shared-expert intermediate. The router runs
replicated on every core; fp32 token tiles for the router matmul are
rebuilt on-chip from a bf16 (value, residual) pair so no fp32 token DMA
is needed. Expert/shared matmuls run in bf16 with fp32 PSUM accumulation.
Per-core partial outputs [H, T] are summed with a chunked ReduceScatter;
core c ends with rows [128c, 128(c+1)) of the summed transposed output.
The host concatenates the 8 shards and transposes back to [B, S, H].

Optimizations vs the first working version (TimelineSim per-core
504us -> 384us, PE engine ~95% busy):
- Weight/token DMAs consolidated into few large contiguous transfers
  (HWDGE descriptor-generation is ~625ns per dma_start regardless of
  size; the old version's 300+ DMAs cost ~190us of HWDGE serial time).
  First-needed data DMAs first; the very first loads are half-split so
  the PE starts at ~3us.
- No fp32 token DMA: router tokens = tokb + tokr (bf16 pair) added
  on the Pool engine; exact to ~2^-18 relative.
- Router group-logic batched per 512-token chunk (wide DVE ops with
  segmented reduces / broadcast compares) instead of ~30 tiny ops per
  128-token tile; router for chunk c+1 overlaps chunk c's down phase,
  with the dependent transpose/wb matmuls deferred into chunk c+1 so
  they never block the PE queue.
- Activation engine locked to the sigmoid table the whole kernel:
  silu(x) computed as x*sigmoid(x) (one extra multiply) so no
  LoadActFuncSet thrash; PSUM->SBUF copies run on the Activation
  engine (Copy shares the sigmoid table set).
- hidden = (silu(g)*u)*wb with the routing weight applied last, on the
  Pool engine, so PSUM bank recycling never waits on the router chain.
- Down-projection PSUM double-buffered; outputs staged per h-tile so
  the store DMA overlaps the remaining down matmuls; the final chunk
  is column-split in two so the last ReduceScatter exposes half the
  data.
"""

import sys

sys.path.insert(0, "/opt/trn_rl_repo")

import numpy as np
import ml_dtypes

from concourse import bass, bacc, mybir, tile
from concourse.bass_utils import run_bass_kernel_spmd

F32 = mybir.dt.float32
BF16 = mybir.dt.bfloat16
AF = mybir.ActivationFunctionType
ALU = mybir.AluOpType
AX = mybir.AxisListType

B, S, H = 2, 1024, 1024
T = B * S                 # 2048 tokens
I = 512                   # moe intermediate
E = 32                    # routed experts
TOP_K = 4
N_GROUP = 4
GRP = E // N_GROUP        # 8 experts per group
TOPK_GROUP = 2
SCALE = 2.5
SH_I = 1024               # shared intermediate (2 * I)
NCORES = 8
E_LOC = E // NCORES       # 4 experts per core
SH_LOC = SH_I // NCORES   # 128 shared-intermediate rows per core

USE_RS = True             # on-device ReduceScatter; False -> host-side sum

P = 128
TC = 512                  # t-chunk (moving free dim)
NT = T // TC              # 4 t-chunks
TT_C = TC // P            # 4 t-tiles per chunk
NH = H // P               # 8 h-tiles
NI = I // P               # 4 i-tiles per expert


def _build(trace: bool = False):
    nc = bacc.Bacc("TRN2", target_bir_lowering=False, debug=False,
                   num_devices=NCORES)

    # ---- kernel I/O (per-core tensors; contents differ per core) ----
    tokb_d = nc.dram_tensor("tokb_d", [NT, P, NH * TC], BF16,
                            kind="ExternalInput")
    tokr_d = nc.dram_tensor("tokr_d", [NT, P, NH * TC], BF16,
                            kind="ExternalInput")
    rw2 = nc.dram_tensor("rw2", [P, NH * E], F32, kind="ExternalInput")
    rbias4 = nc.dram_tensor("rbias4", [1, TT_C * E], F32,
                            kind="ExternalInput")
    ident = nc.dram_tensor("ident", [P, P], F32, kind="ExternalInput")
    selb4 = nc.dram_tensor("selb4", [E_LOC, E, P], BF16, kind="ExternalInput")
    gw2 = nc.dram_tensor("gw2", [E_LOC, P, NH * I], BF16,
                         kind="ExternalInput")
    uw2 = nc.dram_tensor("uw2", [E_LOC, P, NH * I], BF16,
                         kind="ExternalInput")
    dw2 = nc.dram_tensor("dw2", [E_LOC, P, NI * H], BF16,
                         kind="ExternalInput")
    sgw2 = nc.dram_tensor("sgw2", [P, NH * SH_LOC], BF16,
                          kind="ExternalInput")
    suw2 = nc.dram_tensor("suw2", [P, NH * SH_LOC], BF16,
                          kind="ExternalInput")
    sdw2 = nc.dram_tensor("sdw2", [SH_LOC, H], BF16, kind="ExternalInput")
    if USE_RS:
        out_shard = nc.dram_tensor("out_shard", [P, T], F32,
                                   kind="ExternalOutput")
    else:
        out_shard = nc.dram_tensor("out_part", [H, T], F32,
                                   kind="ExternalOutput")

    rg = [list(range(NCORES))]

    with tile.TileContext(nc) as tc:
        with (
            tc.tile_pool(name="wp", bufs=1) as wp,          # resident weights
            tc.tile_pool(name="cp", bufs=1) as cp,          # consts
            tc.tile_pool(name="tp", bufs=2) as tp,          # token chunks
            tc.tile_pool(name="fp", bufs=1) as fp,          # fp32 router toks
            tc.tile_pool(name="hp", bufs=1) as hp,          # hidden (bf16)
            tc.tile_pool(name="xp", bufs=2) as xp,          # f32 work tiles
            tc.tile_pool(name="rr", bufs=2) as rr,          # router smalls
            tc.tile_pool(name="pg", bufs=2, space="PSUM") as pg,
            tc.tile_pool(name="pu", bufs=2, space="PSUM") as pu,
            tc.tile_pool(name="pd", bufs=2, space="PSUM") as pd,
            tc.tile_pool(name="pm", bufs=1, space="PSUM") as pm,   # mix tag
            tc.tile_pool(name="pw", bufs=1, space="PSUM") as pw,   # wb
            tc.tile_pool(name="dram", bufs=1, space="DRAM") as dp,
        ):
            # ---------- DMAs ordered so the first-needed data lands first:
            # chunk-0 tokens -> expert-0 gate/up -> router residual ->
            # small consts -> remaining gate/up -> shared -> down.
            tok_sb = {}
            HNH = NH // 2
            t_ = tp.tile([P, NH, TC], BF16, tag="tokb")
            tok0_3d = tokb_d[0].rearrange("p (h t) -> p h t", h=NH)
            nc.sync.dma_start(t_[:, :HNH, :], tok0_3d[:, :HNH, :])
            gw_sb, uw_sb = {}, {}
            g_ = wp.tile([P, NH * I], BF16, tag="gw0")
            nc.sync.dma_start(g_[:, :HNH * I], gw2[0, :, :HNH * I])
            gw_sb[0] = g_
            u_ = wp.tile([P, NH * I], BF16, tag="uw0")
            nc.sync.dma_start(u_[:, :HNH * I], uw2[0, :, :HNH * I])
            uw_sb[0] = u_
            nc.sync.dma_start(t_[:, HNH:, :], tok0_3d[:, HNH:, :])
            nc.sync.dma_start(g_[:, HNH * I:], gw2[0, :, HNH * I:])
            nc.sync.dma_start(u_[:, HNH * I:], uw2[0, :, HNH * I:])
            r_ = tp.tile([P, NH, TC], BF16, tag="tokr", bufs=1)
            nc.sync.dma_start(r_[:], tokr_d[0].rearrange(
                "p (h t) -> p h t", h=NH))
            tok_sb[0] = (t_, r_)
            ones = cp.tile([1, P], F32, tag="ones")
            nc.vector.memset(ones[:], 1.0)
            ident_sb = cp.tile([P, P], F32, tag="ident")
            nc.sync.dma_start(ident_sb[:], ident[:, :])
            rw_sb = cp.tile([P, NH * E], F32, tag="rw")
            nc.sync.dma_start(rw_sb[:], rw2[:, :])
            rbias_sb = cp.tile([1, TT_C * E], F32, tag="rbias")
            nc.sync.dma_start(rbias_sb[:], rbias4[:, :])
            selb_sb = []
            for el in range(E_LOC):
                s_ = cp.tile([E, P], BF16, tag=f"selb{el}")
                nc.sync.dma_start(s_[:], selb4[el, :, :])
                selb_sb.append(s_)
            for el in range(1, E_LOC):
                g_ = wp.tile([P, NH * I], BF16, tag=f"gw{el}")
                nc.sync.dma_start(g_[:], gw2[el, :, :])
                gw_sb[el] = g_
                u_ = wp.tile([P, NH * I], BF16, tag=f"uw{el}")
                nc.sync.dma_start(u_[:], uw2[el, :, :])
                uw_sb[el] = u_
            sgw_sb = wp.tile([P, NH * SH_LOC], BF16, tag="sgw")
            nc.sync.dma_start(sgw_sb[:], sgw2[:, :])
            suw_sb = wp.tile([P, NH * SH_LOC], BF16, tag="suw")
            nc.sync.dma_start(suw_sb[:], suw2[:, :])
            sdw_sb = wp.tile([SH_LOC, H], BF16, tag="sdw")
            nc.sync.dma_start(sdw_sb[:], sdw2[:, :])
            dw_sb = {}
            for el in range(E_LOC):
                d_ = wp.tile([P, NI * H], BF16, tag=f"dw{el}")
                nc.sync.dma_start(d_[:], dw2[el, :, :])
                dw_sb[el] = d_

            # bias broadcast [P, TT_C*E] via ones^T @ rbias4
            bias_ps = pm.tile([P, TT_C * E], F32, tag="mix")
            nc.tensor.matmul(bias_ps[:], ones[:], rbias_sb[:],
                             start=True, stop=True)
            bias_b = cp.tile([P, TT_C * E], F32, tag="bias_b")
            nc.vector.tensor_copy(bias_b[:], bias_ps[:])

            wfin_c = {}     # per-chunk final routing weights [t, (tt e)]
            wt_sbc = {}     # transposed [(tt e), t], bf16

            def emit_router(c):
                """Router math for chunk c: logits + sigmoid + batched
                top-k on DVE. The dependent transpose is deferred to the
                start of chunk c so it never blocks the PE queue."""
                tokb_c, tokr_c = tok_sb[c]
                # fp32 tokens for exact logits: tf32 = b + r  (Pool engine)
                tf32 = []
                for ht in range(NH):
                    f_ = fp.tile([P, TC], F32, tag=f"tf{ht}")
                    nc.gpsimd.tensor_add(f_[:], tokb_c[:, ht, :],
                                         tokr_c[:, ht, :])
                    tf32.append(f_)
                # logits for the 4 t-tiles accumulate into one PSUM bank,
                # disjoint column slices; single bank-clear on the first.
                lgp = pm.tile([P, TT_C * E], F32, tag="mix")
                for tt in range(TT_C):
                    tsl = slice(tt * P, (tt + 1) * P)
                    for ht in range(NH):
                        nc.tensor.matmul(
                            lgp[:, tt * E:(tt + 1) * E],
                            tf32[ht][:, tsl],
                            rw_sb[:, ht * E:(ht + 1) * E],
                            start=(tt == 0 and ht == 0),
                            stop=(ht == NH - 1),
                            skip_group_check=True)
                scores = rr.tile([P, TT_C * E], F32, tag="scores")
                nc.scalar.activation(scores[:], lgp[:], AF.Sigmoid)

                NSEG = TT_C * N_GROUP  # 16 groups of GRP=8 experts
                seg = lambda ap: ap.rearrange("p (n g) -> p n g", g=GRP)
                grp4 = lambda ap: ap.rearrange("p (t n) -> p t n", n=N_GROUP)
                exp32 = lambda ap: ap.rearrange("p (t e) -> p t e", e=E)
                bc = lambda ap, shp: ap.unsqueeze(-1).broadcast_to(shp)

                sfc = rr.tile([P, TT_C * E], F32, tag="sfc")
                nc.vector.tensor_add(sfc[:], scores[:], bias_b[:])
                # per-group top-2 sum == max + second-max
                m1 = rr.tile([P, NSEG], F32, tag="m1")
                nc.vector.tensor_reduce(m1[:], seg(sfc[:]), axis=AX.X,
                                        op=ALU.max)
                eq1 = rr.tile([P, TT_C * E], F32, tag="eq1")
                nc.vector.tensor_tensor(seg(eq1[:]), seg(sfc[:]),
                                        bc(m1[:], [P, NSEG, GRP]),
                                        op=ALU.is_equal)
                sfc_wo = rr.tile([P, TT_C * E], F32, tag="sfc_wo")
                nc.vector.scalar_tensor_tensor(sfc_wo[:], eq1[:], -1e30,
                                               sfc[:], op0=ALU.mult,
                                               op1=ALU.add)
                m2 = rr.tile([P, NSEG], F32, tag="m2")
                nc.vector.tensor_reduce(m2[:], seg(sfc_wo[:]), axis=AX.X,
                                        op=ALU.max)
                gs = rr.tile([P, NSEG], F32, tag="gs")
                nc.vector.tensor_add(gs[:], m1[:], m2[:])
                # top-2 groups of 4, per t-tile
                gm1 = rr.tile([P, TT_C], F32, tag="gm1")
                nc.vector.tensor_reduce(gm1[:], grp4(gs[:]), axis=AX.X,
                                        op=ALU.max)
                eqg = rr.tile([P, NSEG], F32, tag="eqg")
                nc.vector.tensor_tensor(grp4(eqg[:]), grp4(gs[:]),
                                        bc(gm1[:], [P, TT_C, N_GROUP]),
                                        op=ALU.is_equal)
                gs2 = rr.tile([P, NSEG], F32, tag="gs2")
                nc.vector.scalar_tensor_tensor(gs2[:], eqg[:], -1e30, gs[:],
                                               op0=ALU.mult, op1=ALU.add)
                gm2 = rr.tile([P, TT_C], F32, tag="gm2")
                nc.vector.tensor_reduce(gm2[:], grp4(gs2[:]), axis=AX.X,
                                        op=ALU.max)
                gmask = rr.tile([P, NSEG], F32, tag="gmask")
                nc.vector.tensor_tensor(grp4(gmask[:]), grp4(gs[:]),
                                        bc(gm2[:], [P, TT_C, N_GROUP]),
                                        op=ALU.is_ge)
                masked = rr.tile([P, TT_C * E], F32, tag="masked")
                nc.vector.tensor_tensor(seg(masked[:]), seg(sfc[:]),
                                        bc(gmask[:], [P, NSEG, GRP]),
                                        op=ALU.mult)
                sel = rr.tile([P, TT_C * E], F32, tag="sel")
                nc.vector.memset(sel[:], 0.0)
                for _k in range(TOP_K):
                    mk = rr.tile([P, TT_C], F32, tag="mk")
                    nc.vector.tensor_reduce(mk[:], exp32(masked[:]),
                                            axis=AX.X, op=ALU.max)
                    eqk = rr.tile([P, TT_C * E], F32, tag="eqk")
                    nc.vector.tensor_tensor(exp32(eqk[:]), exp32(masked[:]),
                                            bc(mk[:], [P, TT_C, E]),
                                            op=ALU.is_equal)
                    nc.vector.tensor_add(sel[:], sel[:], eqk[:])
                    nc.vector.scalar_tensor_tensor(masked[:], eqk[:], -1e30,
                                                   masked[:], op0=ALU.mult,
                                                   op1=ALU.add)
                wun = rr.tile([P, TT_C * E], F32, tag="wun")
                nc.vector.tensor_mul(wun[:], scores[:], sel[:])
                den = rr.tile([P, TT_C], F32, tag="den")
                nc.vector.tensor_reduce(den[:], exp32(wun[:]), axis=AX.X,
                                        op=ALU.add)
                rec = rr.tile([P, TT_C], F32, tag="rec")
                nc.vector.reciprocal(rec[:], den[:])
                nc.vector.tensor_scalar_mul(rec[:], rec[:], SCALE)
                wfin = rr.tile([P, TT_C * E], F32, tag="wfin")
                nc.vector.tensor_tensor(exp32(wfin[:]), exp32(wun[:]),
                                        bc(rec[:], [P, TT_C, E]),
                                        op=ALU.mult)
                wfin_c[c] = wfin

            def emit_router_transpose(c):
                # [t, (tt e)] -> [e, (tt t)]: four 128x32 transposes into
                # disjoint column slices of one PSUM bank (single clear).
                wt_ps = pm.tile([E, TC], F32, tag="mix")
                for tt in range(TT_C):
                    nc.tensor.matmul(
                        wt_ps[:, tt * P:(tt + 1) * P],
                        wfin_c[c][:, tt * E:(tt + 1) * E],
                        ident_sb[:], is_transpose=True,
                        start=(tt == 0), stop=True,
                        skip_group_check=True)
                w_ = rr.tile([E, TC], BF16, tag="wt")
                nc.vector.tensor_copy(w_[:], wt_ps[:])
                wt_sbc[c] = w_

            emit_router(0)

            # ---------- main chunk loop ----------
            for c in range(NT):
                tsl = slice(c * TC, (c + 1) * TC)
                tokb_c, _ = tok_sb[c]
                emit_router_transpose(c)
                if c + 1 < NT:
                    t_ = tp.tile([P, NH, TC], BF16, tag="tokb")
                    nc.sync.dma_start(t_[:], tokb_d[c + 1].rearrange(
                        "p (h t) -> p h t", h=NH))
                    r_ = tp.tile([P, NH, TC], BF16, tag="tokr", bufs=1)
                    nc.sync.dma_start(r_[:], tokr_d[c + 1].rearrange(
                        "p (h t) -> p h t", h=NH))
                    tok_sb[c + 1] = (t_, r_)

                # ---- routed experts: gate/up + silu + routing weight ----
                hid = {}
                for el in range(E_LOC):
                    # routing weights broadcast across partitions:
                    # wb[i, t] = wt[(tt, e_gl), t] via per-t-tile selb matmul
                    wb_ps = pw.tile([P, TC], F32, tag="wb_ps")
                    nc.tensor.matmul(wb_ps[:], selb_sb[el][:],
                                     wt_sbc[c][:], start=True, stop=True)
                    wb_sb = xp.tile([P, TC], F32, tag="wb")
                    nc.scalar.copy(wb_sb[:], wb_ps[:])
                    for it in range(NI):
                        g_ps = pg.tile([P, TC], F32, tag="g_ps")
                        u_ps = pu.tile([P, TC], F32, tag="u_ps")
                        for ht in range(NH):
                            nc.tensor.matmul(
                                g_ps[:],
                                gw_sb[el][:, ht * I + it * P:
                                          ht * I + (it + 1) * P],
                                tokb_c[:, ht, :],
                                start=(ht == 0), stop=(ht == NH - 1))
                        for ht in range(NH):
                            nc.tensor.matmul(
                                u_ps[:],
                                uw_sb[el][:, ht * I + it * P:
                                          ht * I + (it + 1) * P],
                                tokb_c[:, ht, :],
                                start=(ht == 0), stop=(ht == NH - 1))
                        sig_g = xp.tile([P, TC], F32, tag="sig")
                        nc.scalar.activation(sig_g[:], g_ps[:], AF.Sigmoid)
                        mul1 = xp.tile([P, TC], F32, tag="mul1")
                        nc.vector.tensor_mul(mul1[:], g_ps[:], sig_g[:])
                        # silu(g)*u frees both PSUM banks without touching
                        # wb, so PSUM recycling never waits on the router
                        mul2 = xp.tile([P, TC], F32, tag="mul2")
                        nc.vector.tensor_mul(mul2[:], mul1[:], u_ps[:])
                        h_ = hp.tile([P, TC], BF16, tag=f"hid{el}_{it}")
                        nc.gpsimd.tensor_mul(h_[:], mul2[:], wb_sb[:])
                        hid[(el, it)] = h_

                # ---- shared expert (weights arrive after gate/up) ----
                sg_ps = pg.tile([P, TC], F32, tag="g_ps")
                su_ps = pu.tile([P, TC], F32, tag="u_ps")
                for ht in range(NH):
                    nc.tensor.matmul(sg_ps[:],
                                     sgw_sb[:, ht * SH_LOC:(ht + 1) * SH_LOC],
                                     tokb_c[:, ht, :],
                                     start=(ht == 0), stop=(ht == NH - 1))
                for ht in range(NH):
                    nc.tensor.matmul(su_ps[:],
                                     suw_sb[:, ht * SH_LOC:(ht + 1) * SH_LOC],
                                     tokb_c[:, ht, :],
                                     start=(ht == 0), stop=(ht == NH - 1))
                sig_s = xp.tile([P, TC], F32, tag="sig")
                nc.scalar.activation(sig_s[:], sg_ps[:], AF.Sigmoid)
                smul = xp.tile([P, TC], F32, tag="mul1")
                nc.vector.tensor_mul(smul[:], sg_ps[:], sig_s[:])
                sh_hid = hp.tile([P, TC], BF16, tag="sh_hid")
                nc.vector.tensor_mul(sh_hid[:], smul[:], su_ps[:])

                # router for the next chunk overlaps this chunk's down phase
                if c + 1 < NT:
                    emit_router(c + 1)

                # ---- down projection; per-h-tile store overlaps compute.
                # The final chunk is split into two column halves so the
                # last (exposed) ReduceScatter covers half the data.
                n_split = 2 if c == NT - 1 else 1
                HC = TC // n_split
                for half in range(n_split):
                    csl = slice(half * HC, half * HC + HC)
                    if USE_RS:
                        cc_in = dp.tile([H, HC], F32, tag=f"cc_in{c}_{half}")
                    for ht in range(NH):
                        hsl_ = slice(ht * P, (ht + 1) * P)
                        d_ps = pd.tile([P, HC], F32, tag="d_ps")
                        k = 0
                        for el in range(E_LOC):
                            for it in range(NI):
                                nc.tensor.matmul(
                                    d_ps[:],
                                    dw_sb[el][:, it * H + ht * P:
                                              it * H + (ht + 1) * P],
                                    hid[(el, it)][:, csl],
                                    start=(k == 0), stop=False)
                                k += 1
                        nc.tensor.matmul(d_ps[:], sdw_sb[:, hsl_],
                                         sh_hid[:, csl],
                                         start=False, stop=True)
                        o_sb = xp.tile([P, HC], F32, tag="o_sb")
                        nc.scalar.copy(o_sb[:], d_ps[:])
                        if USE_RS:
                            nc.sync.dma_start(cc_in[hsl_, :], o_sb[:])
                        else:
                            nc.sync.dma_start(
                                out_shard[hsl_, c * TC + half * HC:
                                          c * TC + half * HC + HC], o_sb[:])
                    if USE_RS:
                        cc_out = dp.tile([P, HC], F32,
                                         tag=f"cc_out{c}_{half}")
                        nc.gpsimd.collective_compute(
                            "ReduceScatter", ALU.add, replica_groups=rg,
                            ins=[cc_in.opt()], outs=[cc_out.opt()],
                        )
                        nc.sync.dma_start(
                            out_shard[:, c * TC + half * HC:
                                      c * TC + half * HC + HC], cc_out[:])

    nc.compile()
    return nc


def _prep_inputs(hidden_states, router_weight, router_bias, gate_w, up_w,
                 down_w, shared_gate_w, shared_up_w, shared_down_w):
    bf = ml_dtypes.bfloat16
    f32 = np.float32

    tokens = np.asarray(hidden_states, dtype=f32).reshape(T, H)
    tokT = np.ascontiguousarray(tokens.T)                       # [H, T]
    tb = tokT.astype(bf)
    tr = (tokT - tb.astype(f32)).astype(bf)

    def chunkize(a):   # [H, T] -> [NT, P, NH*TC] with [c][p][ht][t]
        return np.ascontiguousarray(
            a.reshape(NH, P, NT, TC).transpose(2, 1, 0, 3)
            .reshape(NT, P, NH * TC))

    tokb_d = chunkize(tb)
    tokr_d = chunkize(tr)

    rwT = np.asarray(router_weight, dtype=f32).T                # [H, E]
    rw2 = np.ascontiguousarray(
        rwT.reshape(NH, P, E).transpose(1, 0, 2).reshape(P, NH * E))
    rbias4 = np.tile(np.asarray(router_bias, dtype=f32).reshape(1, E),
                     (1, TT_C))
    ident_np = np.eye(P, dtype=f32)

    gwT = np.asarray(gate_w, dtype=f32).transpose(0, 2, 1)      # [E, H, I]
    uwT = np.asarray(up_w, dtype=f32).transpose(0, 2, 1)
    dwT = np.asarray(down_w, dtype=f32).transpose(0, 2, 1)      # [E, I, H]
    sgwT = np.asarray(shared_gate_w, dtype=f32).T               # [H, SH_I]
    suwT = np.asarray(shared_up_w, dtype=f32).T
    sdwT = np.asarray(shared_down_w, dtype=f32).T               # [SH_I, H]

    def pack_w(a, n_outer, inner):  # [n_outer*P, inner] -> [P, n_outer*inner]
        return np.ascontiguousarray(
            a.reshape(n_outer, P, inner).transpose(1, 0, 2)
            .reshape(P, n_outer * inner))

    in_maps = []
    for cidx in range(NCORES):
        ssl = slice(cidx * SH_LOC, (cidx + 1) * SH_LOC)
        sel = np.zeros((E_LOC, E, P), dtype=bf)
        for el in range(E_LOC):
            sel[el, cidx * E_LOC + el, :] = 1.0
        gw_c = np.stack([pack_w(gwT[cidx * E_LOC + el].astype(bf), NH, I)
                         for el in range(E_LOC)])
        uw_c = np.stack([pack_w(uwT[cidx * E_LOC + el].astype(bf), NH, I)
                         for el in range(E_LOC)])
        dw_c = np.stack([pack_w(dwT[cidx * E_LOC + el].astype(bf), NI, H)
                         for el in range(E_LOC)])
        in_maps.append({
            "tokb_d": tokb_d,
            "tokr_d": tokr_d,
            "rw2": rw2,
            "rbias4": rbias4,
            "ident": ident_np,
            "selb4": sel,
            "gw2": gw_c,
            "uw2": uw_c,
            "dw2": dw_c,
            "sgw2": pack_w(sgwT[:, ssl].astype(bf), NH, SH_LOC),
            "suw2": pack_w(suwT[:, ssl].astype(bf), NH, SH_LOC),
            "sdw2": np.ascontiguousarray(sdwT[ssl, :]).astype(bf),
        })
    return in_maps


def run_on_device(inputs: dict, trace: bool = False):
    in_maps = _prep_inputs(**inputs)
    nc = _build(trace=trace)
    res = run_bass_kernel_spmd(nc, in_maps, list(range(NCORES)), trace=trace)
    if USE_RS:
        shards = [res.results[c]["out_shard"] for c in range(NCORES)]
        outT = np.concatenate(shards, axis=0)                   # [H, T]
    else:
        outT = np.sum([res.results[c]["out_part"] for c in range(NCORES)],
                      axis=0, dtype=np.float32)
    out = np.ascontiguousarray(outT.T).reshape(B, S, H).astype(np.float32)
    return out, res


def kernel(**inputs) -> np.ndarray:
    out, _ = run_on_device(inputs, trace=False)
    return out


# revision 19
# speedup vs baseline: 1.0002x; 1.0002x over previous
"""Kimi-K2.5 tensorized MoE kernel for 8 TRN2 NeuronCores.

Sharding: expert-parallel. Core c owns routed experts [4c, 4c+4) and rows
[128c, 128(c+1)) of the shared-expert intermediate. The router runs
replicated on every core; fp32 token tiles for the router matmul are
rebuilt on-chip from a bf16 (value, residual) pair so no fp32 token DMA
is needed. Expert/shared matmuls run in bf16 with fp32 PSUM accumulation.
Per-core partial outputs [H, T] are summed with a chunked ReduceScatter;
core c ends with rows [128c, 128(c+1)) of the summed transposed output.
The host concatenates the 8 shards and transposes back to [B, S, H].

Optimizations vs the first working version (TimelineSim per-core
504us -> 384us, PE engine ~95% busy):
- Weight/token DMAs consolidated into few large contiguous transfers
  (HWDGE descriptor-generation is ~625ns per dma_start regardless of
  size; the old version's 300+ DMAs cost ~190us of HWDGE serial time).
  First-needed data DMAs first; the very first loads are half-split so
  the PE starts at ~3us.
- No fp32 token DMA: router tokens = tokb + tokr (bf16 pair) added
  on the Pool engine; exact to ~2^-18 relative.
- Router group-logic batched per 512-token chunk (wide DVE ops with
  segmented reduces / broadcast compares) instead of ~30 tiny ops per
  128-token tile; router for chunk c+1 overlaps chunk c's down phase,
  with the dependent transpose/wb matmuls deferred into chunk c+1 so
  they never block the PE queue.
- Activation engine locked to the sigmoid table the whole kernel:
  silu(x) computed as x*sigmoid(x) (one extra multiply) so no
  LoadActFuncSet thrash; PSUM->SBUF copies run on the Activation
  engine (Copy shares the sigmoid table set).
- hidden = (silu(g)*u)*wb with the routing weight applied last, on the
  Pool engine, so PSUM bank recycling never waits on the router chain.
- Down-projection PSUM double-buffered; outputs staged per h-tile so
  the store DMA overlaps the remaining down matmuls; the final chunk
  is column-split in two so the last ReduceScatter exposes half the
  data.
"""

import sys

sys.path.insert(0, "/opt/trn_rl_repo")

import numpy as np
import ml_dtypes

from concourse import bass, bacc, mybir, tile
from concourse.bass_utils import run_bass_kernel_spmd

F32 = mybir.dt.float32
BF16 = mybir.dt.bfloat16
AF = mybir.ActivationFunctionType
ALU = mybir.AluOpType
AX = mybir.AxisListType

B, S, H = 2, 1024, 1024
T = B * S                 # 2048 tokens
I = 512                   # moe intermediate
E = 32                    # routed experts
TOP_K = 4
N_GROUP = 4
GRP = E // N_GROUP        # 8 experts per group
TOPK_GROUP = 2
SCALE = 2.5
SH_I = 1024               # shared intermediate (2 * I)
NCORES = 8
E_LOC = E // NCORES       # 4 experts per core
SH_LOC = SH_I // NCORES   # 128 shared-intermediate rows per core

USE_RS = True             # on-device ReduceScatter; False -> host-side sum

P = 128
TC = 512                  # t-chunk (moving free dim)
NT = T // TC              # 4 t-chunks
TT_C = TC // P            # 4 t-tiles per chunk
NH = H // P               # 8 h-tiles
NI = I // P               # 4 i-tiles per expert


def _build(trace: bool = False):
    nc = bacc.Bacc("TRN2", target_bir_lowering=False, debug=False,
                   num_devices=NCORES)

    # ---- kernel I/O (per-core tensors; contents differ per core) ----
    tokb_d = nc.dram_tensor("tokb_d", [NT, P, NH * TC], BF16,
                            kind="ExternalInput")
    tokr_d = nc.dram_tensor("tokr_d", [NT, P, NH * TC], BF16,
                            kind="ExternalInput")
    rw2 = nc.dram_tensor("rw2", [P, NH * E], F32, kind="ExternalInput")
    rbias4 = nc.dram_tensor("rbias4", [1, TT_C * E], F32,
                            kind="ExternalInput")
    ident = nc.dram_tensor("ident", [P, P], F32, kind="ExternalInput")
    selb4 = nc.dram_tensor("selb4", [E_LOC, E, P], BF16, kind="ExternalInput")
    gw2 = nc.dram_tensor("gw2", [E_LOC, P, NH * I], BF16,
                         kind="ExternalInput")
    uw2 = nc.dram_tensor("uw2", [E_LOC, P, NH * I], BF16,
                         kind="ExternalInput")
    dw2 = nc.dram_tensor("dw2", [E_LOC, P, NI * H], BF16,
                         kind="ExternalInput")
    sgw2 = nc.dram_tensor("sgw2", [P, NH * SH_LOC], BF16,
                          kind="ExternalInput")
    suw2 = nc.dram_tensor("suw2", [P, NH * SH_LOC], BF16,
                          kind="ExternalInput")
    sdw2 = nc.dram_tensor("sdw2", [SH_LOC, H], BF16, kind="ExternalInput")
    if USE_RS:
        out_shard = nc.dram_tensor("out_shard", [P, T], F32,
                                   kind="ExternalOutput")
    else:
        out_shard = nc.dram_tensor("out_part", [H, T], F32,
                                   kind="ExternalOutput")

    rg = [list(range(NCORES))]

    with tile.TileContext(nc) as tc:
        with (
            tc.tile_pool(name="wp", bufs=1) as wp,          # resident weights
            tc.tile_pool(name="cp", bufs=1) as cp,          # consts
            tc.tile_pool(name="tp", bufs=2) as tp,          # token chunks
            tc.tile_pool(name="fp", bufs=1) as fp,          # fp32 router toks
            tc.tile_pool(name="hp", bufs=1) as hp,          # hidden (bf16)
            tc.tile_pool(name="xp", bufs=2) as xp,          # f32 work tiles
            tc.tile_pool(name="rr", bufs=2) as rr,          # router smalls
            tc.tile_pool(name="pg", bufs=2, space="PSUM") as pg,
            tc.tile_pool(name="pu", bufs=2, space="PSUM") as pu,
            tc.tile_pool(name="pd", bufs=2, space="PSUM") as pd,
            tc.tile_pool(name="pm", bufs=1, space="PSUM") as pm,   # mix tag
            tc.tile_pool(name="pw", bufs=1, space="PSUM") as pw,   # wb
            tc.tile_pool(name="dram", bufs=1, space="DRAM") as dp,
        ):
            # ---------- DMAs ordered so the first-needed data lands first:
            # chunk-0 tokens -> expert-0 gate/up -> router residual ->
            # small consts -> remaining gate/up -> shared -> down.
            tok_sb = {}
            QNH = NH // 4
            t_ = tp.tile([P, NH, TC], BF16, tag="tokb")
            tok0_3d = tokb_d[0].rearrange("p (h t) -> p h t", h=NH)
            gw_sb, uw_sb = {}, {}
            g_ = wp.tile([P, NH * I], BF16, tag="gw0")
            gw_sb[0] = g_
            u_ = wp.tile([P, NH * I], BF16, tag="uw0")
            uw_sb[0] = u_
            for q in range(4):
                hq = slice(q * QNH, (q + 1) * QNH)
                cq = slice(q * QNH * I, (q + 1) * QNH * I)
                nc.sync.dma_start(t_[:, hq, :], tok0_3d[:, hq, :])
                nc.sync.dma_start(g_[:, cq], gw2[0, :, cq])
                nc.sync.dma_start(u_[:, cq], uw2[0, :, cq])
            r_ = tp.tile([P, NH, TC], BF16, tag="tokr", bufs=1)
            nc.sync.dma_start(r_[:], tokr_d[0].rearrange(
                "p (h t) -> p h t", h=NH))
            tok_sb[0] = (t_, r_)
            ones = cp.tile([1, P], F32, tag="ones")
            nc.vector.memset(ones[:], 1.0)
            ident_sb = cp.tile([P, P], F32, tag="ident")
            nc.sync.dma_start(ident_sb[:], ident[:, :])
            rw_sb = cp.tile([P, NH * E], F32, tag="rw")
            nc.sync.dma_start(rw_sb[:], rw2[:, :])
            rbias_sb = cp.tile([1, TT_C * E], F32, tag="rbias")
            nc.sync.dma_start(rbias_sb[:], rbias4[:, :])
            selb_sb = []
            for el in range(E_LOC):
                s_ = cp.tile([E, P], BF16, tag=f"selb{el}")
                nc.sync.dma_start(s_[:], selb4[el, :, :])
                selb_sb.append(s_)
            for el in range(1, E_LOC):
                g_ = wp.tile([P, NH * I], BF16, tag=f"gw{el}")
                nc.sync.dma_start(g_[:], gw2[el, :, :])
                gw_sb[el] = g_
                u_ = wp.tile([P, NH * I], BF16, tag=f"uw{el}")
                nc.sync.dma_start(u_[:], uw2[el, :, :])
                uw_sb[el] = u_
            sgw_sb = wp.tile([P, NH * SH_LOC], BF16, tag="sgw")
            nc.sync.dma_start(sgw_sb[:], sgw2[:, :])
            suw_sb = wp.tile([P, NH * SH_LOC], BF16, tag="suw")
            nc.sync.dma_start(suw_sb[:], suw2[:, :])
            sdw_sb = wp.tile([SH_LOC, H], BF16, tag="sdw")
            nc.sync.dma_start(sdw_sb[:], sdw2[:, :])
            dw_sb = {}
            for el in range(E_LOC):
                d_ = wp.tile([P, NI * H], BF16, tag=f"dw{el}")
                nc.sync.dma_start(d_[:], dw2[el, :, :])
                dw_sb[el] = d_

            # bias broadcast [P, TT_C*E] via ones^T @ rbias4
            bias_ps = pm.tile([P, TT_C * E], F32, tag="mix")
            nc.tensor.matmul(bias_ps[:], ones[:], rbias_sb[:],
                             start=True, stop=True)
            bias_b = cp.tile([P, TT_C * E], F32, tag="bias_b")
            nc.vector.tensor_copy(bias_b[:], bias_ps[:])

            wfin_c = {}     # per-chunk final routing weights [t, (tt e)]
            wt_sbc = {}     # transposed [(tt e), t], bf16

            def emit_router(c):
                """Router math for chunk c: logits + sigmoid + batched
                top-k on DVE. The dependent transpose is deferred to the
                start of chunk c so it never blocks the PE queue."""
                tokb_c, tokr_c = tok_sb[c]
                # fp32 tokens for exact logits: tf32 = b + r  (Pool engine)
                tf32 = []
                for ht in range(NH):
                    f_ = fp.tile([P, TC], F32, tag=f"tf{ht}")
                    nc.gpsimd.tensor_add(f_[:], tokb_c[:, ht, :],
                                         tokr_c[:, ht, :])
                    tf32.append(f_)
                # logits for the 4 t-tiles accumulate into one PSUM bank,
                # disjoint column slices; single bank-clear on the first.
                lgp = pm.tile([P, TT_C * E], F32, tag="mix")
                for tt in range(TT_C):
                    tsl = slice(tt * P, (tt + 1) * P)
                    for ht in range(NH):
                        nc.tensor.matmul(
                            lgp[:, tt * E:(tt + 1) * E],
                            tf32[ht][:, tsl],
                            rw_sb[:, ht * E:(ht + 1) * E],
                            start=(tt == 0 and ht == 0),
                            stop=(ht == NH - 1),
                            skip_group_check=True)
                scores = rr.tile([P, TT_C * E], F32, tag="scores")
                nc.scalar.activation(scores[:], lgp[:], AF.Sigmoid)

                NSEG = TT_C * N_GROUP  # 16 groups of GRP=8 experts
                seg = lambda ap: ap.rearrange("p (n g) -> p n g", g=GRP)
                grp4 = lambda ap: ap.rearrange("p (t n) -> p t n", n=N_GROUP)
                exp32 = lambda ap: ap.rearrange("p (t e) -> p t e", e=E)
                bc = lambda ap, shp: ap.unsqueeze(-1).broadcast_to(shp)

                sfc = rr.tile([P, TT_C * E], F32, tag="sfc")
                nc.vector.tensor_add(sfc[:], scores[:], bias_b[:])
                # per-group top-2 sum == max + second-max
                m1 = rr.tile([P, NSEG], F32, tag="m1")
                nc.vector.tensor_reduce(m1[:], seg(sfc[:]), axis=AX.X,
                                        op=ALU.max)
                eq1 = rr.tile([P, TT_C * E], F32, tag="eq1")
                nc.vector.tensor_tensor(seg(eq1[:]), seg(sfc[:]),
                                        bc(m1[:], [P, NSEG, GRP]),
                                        op=ALU.is_equal)
                sfc_wo = rr.tile([P, TT_C * E], F32, tag="sfc_wo")
                nc.vector.scalar_tensor_tensor(sfc_wo[:], eq1[:], -1e30,
                                               sfc[:], op0=ALU.mult,
                                               op1=ALU.add)
                m2 = rr.tile([P, NSEG], F32, tag="m2")
                nc.vector.tensor_reduce(m2[:], seg(sfc_wo[:]), axis=AX.X,
                                        op=ALU.max)
                gs = rr.tile([P, NSEG], F32, tag="gs")
                nc.vector.tensor_add(gs[:], m1[:], m2[:])
                # top-2 groups of 4, per t-tile
                gm1 = rr.tile([P, TT_C], F32, tag="gm1")
                nc.vector.tensor_reduce(gm1[:], grp4(gs[:]), axis=AX.X,
                                        op=ALU.max)
                eqg = rr.tile([P, NSEG], F32, tag="eqg")
                nc.vector.tensor_tensor(grp4(eqg[:]), grp4(gs[:]),
                                        bc(gm1[:], [P, TT_C, N_GROUP]),
                                        op=ALU.is_equal)
                gs2 = rr.tile([P, NSEG], F32, tag="gs2")
                nc.vector.scalar_tensor_tensor(gs2[:], eqg[:], -1e30, gs[:],
                                               op0=ALU.mult, op1=ALU.add)
                gm2 = rr.tile([P, TT_C], F32, tag="gm2")
                nc.vector.tensor_reduce(gm2[:], grp4(gs2[:]), axis=AX.X,
                                        op=ALU.max)
                gmask = rr.tile([P, NSEG], F32, tag="gmask")
                nc.vector.tensor_tensor(grp4(gmask[:]), grp4(gs[:]),
                                        bc(gm2[:], [P, TT_C, N_GROUP]),
                                        op=ALU.is_ge)
                masked = rr.tile([P, TT_C * E], F32, tag="masked")
                nc.vector.tensor_tensor(seg(masked[:]), seg(sfc[:]),
                                        bc(gmask[:], [P, NSEG, GRP]),
                                        op=ALU.mult)
                sel = rr.tile([P, TT_C * E], F32, tag="sel")
                nc.vector.memset(sel[:], 0.0)
                for _k in range(TOP_K):
                    mk = rr.tile([P, TT_C], F32, tag="mk")
                    nc.vector.tensor_reduce(mk[:], exp32(masked[:]),
                                            axis=AX.X, op=ALU.max)
                    eqk = rr.tile([P, TT_C * E], F32, tag="eqk")
                    nc.vector.tensor_tensor(exp32(eqk[:]), exp32(masked[:]),
                                            bc(mk[:], [P, TT_C, E]),
                                            op=ALU.is_equal)
                    nc.vector.tensor_add(sel[:], sel[:], eqk[:])
                    nc.vector.scalar_tensor_tensor(masked[:], eqk[:], -1e30,
                                                   masked[:], op0=ALU.mult,
                                                   op1=ALU.add)
                wun = rr.tile([P, TT_C * E], F32, tag="wun")
                nc.vector.tensor_mul(wun[:], scores[:], sel[:])
                den = rr.tile([P, TT_C], F32, tag="den")
                nc.vector.tensor_reduce(den[:], exp32(wun[:]), axis=AX.X,
                                        op=ALU.add)
                rec = rr.tile([P, TT_C], F32, tag="rec")
                nc.vector.reciprocal(rec[:], den[:])
                nc.vector.tensor_scalar_mul(rec[:], rec[:], SCALE)
                wfin = rr.tile([P, TT_C * E], F32, tag="wfin")
                nc.vector.tensor_tensor(exp32(wfin[:]), exp32(wun[:]),
                                        bc(rec[:], [P, TT_C, E]),
                                        op=ALU.mult)
                wfin_c[c] = wfin

            def emit_router_transpose(c):
                # [t, (tt e)] -> [e, (tt t)]: four 128x32 transposes into
                # disjoint column slices of one PSUM bank (single clear).
                wt_ps = pm.tile([E, TC], F32, tag="mix")
                for tt in range(TT_C):
                    nc.tensor.matmul(
                        wt_ps[:, tt * P:(tt + 1) * P],
                        wfin_c[c][:, tt * E:(tt + 1) * E],
                        ident_sb[:], is_transpose=True,
                        start=(tt == 0), stop=True,
                        skip_group_check=True)
                w_ = rr.tile([E, TC], BF16, tag="wt")
                nc.vector.tensor_copy(w_[:], wt_ps[:])
                wt_sbc[c] = w_

            emit_router(0)

            # ---------- main chunk loop ----------
            for c in range(NT):
                tsl = slice(c * TC, (c + 1) * TC)
                tokb_c, _ = tok_sb[c]
                emit_router_transpose(c)
                if c + 1 < NT:
                    t_ = tp.tile([P, NH, TC], BF16, tag="tokb")
                    nc.sync.dma_start(t_[:], tokb_d[c + 1].rearrange(
                        "p (h t) -> p h t", h=NH))
                    r_ = tp.tile([P, NH, TC], BF16, tag="tokr", bufs=1)
                    nc.sync.dma_start(r_[:], tokr_d[c + 1].rearrange(
                        "p (h t) -> p h t", h=NH))
                    tok_sb[c + 1] = (t_, r_)

                # ---- routed experts: gate/up + silu + routing weight ----
                hid = {}
                for el in range(E_LOC):
                    # routing weights broadcast across partitions:
                    # wb[i, t] = wt[(tt, e_gl), t] via per-t-tile selb matmul
                    wb_ps = pw.tile([P, TC], F32, tag="wb_ps")
                    nc.tensor.matmul(wb_ps[:], selb_sb[el][:],
                                     wt_sbc[c][:], start=True, stop=True)
                    wb_sb = xp.tile([P, TC], F32, tag="wb")
                    nc.scalar.copy(wb_sb[:], wb_ps[:])
                    for it in range(NI):
                        g_ps = pg.tile([P, TC], F32, tag="g_ps")
                        u_ps = pu.tile([P, TC], F32, tag="u_ps")
                        for ht in range(NH):
                            nc.tensor.matmul(
                                g_ps[:],
                                gw_sb[el][:, ht * I + it * P:
                                          ht * I + (it + 1) * P],
                                tokb_c[:, ht, :],
                                start=(ht == 0), stop=(ht == NH - 1))
                        for ht in range(NH):
                            nc.tensor.matmul(
                                u_ps[:],
                                uw_sb[el][:, ht * I + it * P:
                                          ht * I + (it + 1) * P],
                                tokb_c[:, ht, :],
                                start=(ht == 0), stop=(ht == NH - 1))
                        sig_g = xp.tile([P, TC], F32, tag="sig")
                        nc.scalar.activation(sig_g[:], g_ps[:], AF.Sigmoid)
                        mul1 = xp.tile([P, TC], F32, tag="mul1")
                        nc.vector.tensor_mul(mul1[:], g_ps[:], sig_g[:])
                        # silu(g)*u frees both PSUM banks without touching
                        # wb, so PSUM recycling never waits on the router
                        mul2 = xp.tile([P, TC], F32, tag="mul2")
                        nc.vector.tensor_mul(mul2[:], mul1[:], u_ps[:])
                        h_ = hp.tile([P, TC], BF16, tag=f"hid{el}_{it}")
                        nc.gpsimd.tensor_mul(h_[:], mul2[:], wb_sb[:])
                        hid[(el, it)] = h_

                # ---- shared expert (weights arrive after gate/up) ----
                sg_ps = pg.tile([P, TC], F32, tag="g_ps")
                su_ps = pu.tile([P, TC], F32, tag="u_ps")
                for ht in range(NH):
                    nc.tensor.matmul(sg_ps[:],
                                     sgw_sb[:, ht * SH_LOC:(ht + 1) * SH_LOC],
                                     tokb_c[:, ht, :],
                                     start=(ht == 0), stop=(ht == NH - 1))
                for ht in range(NH):
                    nc.tensor.matmul(su_ps[:],
                                     suw_sb[:, ht * SH_LOC:(ht + 1) * SH_LOC],
                                     tokb_c[:, ht, :],
                                     start=(ht == 0), stop=(ht == NH - 1))
                sig_s = xp.tile([P, TC], F32, tag="sig")
                nc.scalar.activation(sig_s[:], sg_ps[:], AF.Sigmoid)
                smul = xp.tile([P, TC], F32, tag="mul1")
                nc.vector.tensor_mul(smul[:], sg_ps[:], sig_s[:])
                sh_hid = hp.tile([P, TC], BF16, tag="sh_hid")
                nc.vector.tensor_mul(sh_hid[:], smul[:], su_ps[:])

                # router for the next chunk overlaps this chunk's down phase
                if c + 1 < NT:
                    emit_router(c + 1)

                # ---- down projection; per-h-tile store overlaps compute.
                # The final chunk is split into two column halves so the
                # last (exposed) ReduceScatter covers half the data.
                n_split = 2 if c == NT - 1 else 1
                HC = TC // n_split
                for half in range(n_split):
                    csl = slice(half * HC, half * HC + HC)
                    if USE_RS:
                        cc_in = dp.tile([H, HC], F32, tag=f"cc_in{c}_{half}")
                    for ht in range(NH):
                        hsl_ = slice(ht * P, (ht + 1) * P)
                        d_ps = pd.tile([P, HC], F32, tag="d_ps")
                        k = 0
                        for el in range(E_LOC):
                            for it in range(NI):
                                nc.tensor.matmul(
                                    d_ps[:],
                                    dw_sb[el][:, it * H + ht * P:
                                              it * H + (ht + 1) * P],
                                    hid[(el, it)][:, csl],
                                    start=(k == 0), stop=False)
                                k += 1
                        nc.tensor.matmul(d_ps[:], sdw_sb[:, hsl_],
                                         sh_hid[:, csl],
                                         start=False, stop=True)
                        o_sb = xp.tile([P, HC], F32, tag="o_sb")
                        nc.scalar.copy(o_sb[:], d_ps[:])
                        if USE_RS:
                            nc.sync.dma_start(cc_in[hsl_, :], o_sb[:])
                        else:
                            nc.sync.dma_start(
                                out_shard[hsl_, c * TC + half * HC:
                                          c * TC + half * HC + HC], o_sb[:])
                    if USE_RS:
                        cc_out = dp.tile([P, HC], F32,
                                         tag=f"cc_out{c}_{half}")
                        nc.gpsimd.collective_compute(
                            "ReduceScatter", ALU.add, replica_groups=rg,
                            ins=[cc_in.opt()], outs=[cc_out.opt()],
                        )
                        nc.sync.dma_start(
                            out_shard[:, c * TC + half * HC:
                                      c * TC + half * HC + HC], cc_out[:])

    nc.compile()
    return nc


def _prep_inputs(hidden_states, router_weight, router_bias, gate_w, up_w,
                 down_w, shared_gate_w, shared_up_w, shared_down_w):
    bf = ml_dtypes.bfloat16
    f32 = np.float32

    tokens = np.asarray(hidden_states, dtype=f32).reshape(T, H)
    tokT = np.ascontiguousarray(tokens.T)                       # [H, T]
    tb = tokT.astype(bf)
    tr = (tokT - tb.astype(f32)).astype(bf)

    def chunkize(a):   # [H, T] -> [NT, P, NH*TC] with [c][p][ht][t]
        return np.ascontiguousarray(
            a.reshape(NH, P, NT, TC).transpose(2, 1, 0, 3)
            .reshape(NT, P, NH * TC))

    tokb_d = chunkize(tb)
    tokr_d = chunkize(tr)

    rwT = np.asarray(router_weight, dtype=f32).T                # [H, E]
    rw2 = np.ascontiguousarray(
        rwT.reshape(NH, P, E).transpose(1, 0, 2).reshape(P, NH * E))
    rbias4 = np.tile(np.asarray(router_bias, dtype=f32).reshape(1, E),
                     (1, TT_C))
    ident_np = np.eye(P, dtype=f32)

    gwT = np.asarray(gate_w, dtype=f32).transpose(0, 2, 1)      # [E, H, I]
    uwT = np.asarray(up_w, dtype=f32).transpose(0, 2, 1)
    dwT = np.asarray(down_w, dtype=f32).transpose(0, 2, 1)      # [E, I, H]
    sgwT = np.asarray(shared_gate_w, dtype=f32).T               # [H, SH_I]
    suwT = np.asarray(shared_up_w, dtype=f32).T
    sdwT = np.asarray(shared_down_w, dtype=f32).T               # [SH_I, H]

    def pack_w(a, n_outer, inner):  # [n_outer*P, inner] -> [P, n_outer*inner]
        return np.ascontiguousarray(
            a.reshape(n_outer, P, inner).transpose(1, 0, 2)
            .reshape(P, n_outer * inner))

    in_maps = []
    for cidx in range(NCORES):
        ssl = slice(cidx * SH_LOC, (cidx + 1) * SH_LOC)
        sel = np.zeros((E_LOC, E, P), dtype=bf)
        for el in range(E_LOC):
            sel[el, cidx * E_LOC + el, :] = 1.0
        gw_c = np.stack([pack_w(gwT[cidx * E_LOC + el].astype(bf), NH, I)
                         for el in range(E_LOC)])
        uw_c = np.stack([pack_w(uwT[cidx * E_LOC + el].astype(bf), NH, I)
                         for el in range(E_LOC)])
        dw_c = np.stack([pack_w(dwT[cidx * E_LOC + el].astype(bf), NI, H)
                         for el in range(E_LOC)])
        in_maps.append({
            "tokb_d": tokb_d,
            "tokr_d": tokr_d,
            "rw2": rw2,
            "rbias4": rbias4,
            "ident": ident_np,
            "selb4": sel,
            "gw2": gw_c,
            "uw2": uw_c,
            "dw2": dw_c,
            "sgw2": pack_w(sgwT[:, ssl].astype(bf), NH, SH_LOC),
            "suw2": pack_w(suwT[:, ssl].astype(bf), NH, SH_LOC),
            "sdw2": np.ascontiguousarray(sdwT[ssl, :]).astype(bf),
        })
    return in_maps


def run_on_device(inputs: dict, trace: bool = False):
    in_maps = _prep_inputs(**inputs)
    nc = _build(trace=trace)
    res = run_bass_kernel_spmd(nc, in_maps, list(range(NCORES)), trace=trace)
    if USE_RS:
        shards = [res.results[c]["out_shard"] for c in range(NCORES)]
        outT = np.concatenate(shards, axis=0)                   # [H, T]
    else:
        outT = np.sum([res.results[c]["out_part"] for c in range(NCORES)],
                      axis=0, dtype=np.float32)
    out = np.ascontiguousarray(outT.T).reshape(B, S, H).astype(np.float32)
    return out, res


def kernel(**inputs) -> np.ndarray:
    out, _ = run_on_device(inputs, trace=False)
    return out
